# revision 44
# baseline (speedup 1.0000x reference)
"""Trainium2 Bass kernel for ExplicitComplexityFusion (segment_reduce).

Computes, for a batch of B=512 graphs packed into N=200000 nodes (batch ids
sorted) and E=6400000 random edges:
    n_nodes[b], n_edges[b] (intra-graph edges), per-graph sigmoid gate
    w_global/w_local, and x_fused = w_local[batch]*x_ggnn + w_global[batch]*x_appnp.

Sharding (per the data-parallel-over-graphs scheme): 64 consecutive graphs per
core; node rows and edges (keyed by src's graph) are routed to the owning core
on the host, so all segment reductions and the fusion are core-local.

Device algorithm per core:
  - histogram of 64 local graph keys via one-hot(hi3)⊗one-hot(lo3) TensorE
    matmuls accumulated in PSUM (edges masked by src_graph==dst_graph),
  - per-graph gate math (ln / reciprocal / sigmoid) on [8,8] tiles,
  - per-node weight = transpose(one-hot64) @ w  on TensorE,
  - fusion via ACT per-partition scale + DVE fused multiply-add.
"""

import math
import os
import sys

import numpy as np

for _p in ("/opt/trn_rl_repo", "/opt/trn_rl_repo/concourse"):
    if os.path.isdir(_p) and _p not in sys.path:
        sys.path.insert(0, _p)

import concourse.bass as bass
import concourse.mybir as mybir
from concourse import bacc
from concourse.masks import make_identity
from concourse.bass_utils import run_bass_kernel_spmd
from concourse.tile import TileContext

F32 = mybir.dt.float32
I16 = mybir.dt.int16
I32 = mybir.dt.int32
BF16 = mybir.dt.bfloat16
AOT = mybir.AluOpType
ACTF = mybir.ActivationFunctionType

N_NODES = 200000
D = 256
N_EDGES = 6400000
BATCH = 512
MAX_NODES = 500
NCORES = 8
GPC = BATCH // NCORES  # graphs per core = 64
TC = 512  # edge columns per chunk
INV_LOG_NORM = 1.0 / math.log(MAX_NODES + 1)

PAD_KEY = 64  # out-of-range key: hi3 = 8 -> zero one-hot row -> no contribution


def _emit_hist_mms(nc, ph, U3, V3, ncols, col0, total_cols, free_off):
    """Accumulate sum_e onehot8(hi)⊗onehot8(lo) into PSUM `ph`.

    U3/V3: [128, ncols, 8] bf16 one-hot tiles (128 edges per column).
    Columns are spread over the 4 PE column-groups via tile_position so up
    to 4 matmuls run concurrently in disjoint 32-partition strips.
    """
    assert ncols % 4 == 0
    for t in range(ncols):
        col = col0 + t
        j = col % 4
        nc.tensor.matmul(
            ph[32 * j : 32 * j + 8, free_off : free_off + 8],
            U3[:, :, t],
            V3[:, :, t],
            start=(col < 4),
            stop=(col >= total_cols - 4),
            tile_position=(0, 32 * j),
        )


def _combine_hist(nc, pool, phs, free_off):
    """Sum the 4 column-group 8x8 blocks of the histogram (SBUF copy).

    Engine ops need equal base partitions for two SBUF inputs, so stage the
    four blocks at base partition 0 first.
    """
    stk = pool.tile([8, 4, 8], F32, tag=f"histstk{free_off}")
    for j in range(4):
        nc.vector.tensor_copy(stk[:, j, :],
                              phs[32 * j : 32 * j + 8, free_off : free_off + 8])
    acc = pool.tile([8, 8], F32, tag=f"histacc{free_off}")
    nc.vector.tensor_tensor(acc[:], stk[:, 0, :], stk[:, 1, :], AOT.add)
    nc.vector.tensor_tensor(acc[:], acc[:], stk[:, 2, :], AOT.add)
    nc.vector.tensor_tensor(acc[:], acc[:], stk[:, 3, :], AOT.add)
    return acc


def build_kernel(NT, ETC):
    """Build the SPMD Bass program. NT: node columns (nodes_pad=128*NT),
    ETC: edge columns (edges_pad=128*ETC). NT % 4 == 0, ETC % TC == 0."""
    # Bacc (not plain Bass): its finalize() legalizes sync waits (split into
    # EventSemaphore chains) -- walrus allows only one wait per instruction.
    nc = bacc.Bacc("TRN2", target_bir_lowering=False)

    xg = nc.dram_tensor("xg", [NT * 128, D], F32, kind="ExternalInput")
    xa = nc.dram_tensor("xa", [NT * 128, D], F32, kind="ExternalInput")
    ekp = nc.dram_tensor("ekp", [128, ETC], I16, kind="ExternalInput")
    ekf = nc.dram_tensor("ekf", [128, ETC], I16, kind="ExternalInput")
    ed = nc.dram_tensor("ed", [128, ETC], I16, kind="ExternalInput")
    nkp = nc.dram_tensor("nkp", [128, NT], I16, kind="ExternalInput")
    nkf8 = nc.dram_tensor("nkf8", [128, NT], I16, kind="ExternalInput")
    abg = nc.dram_tensor("abg", [8, 3], F32, kind="ExternalInput")
    ox = nc.dram_tensor("ox", [NT * 128, D], F32, kind="ExternalOutput")
    owl = nc.dram_tensor("owl", [8, 8], F32, kind="ExternalOutput")
    owg = nc.dram_tensor("owg", [8, 8], F32, kind="ExternalOutput")

    # [row, d] with row = 128*t + p  ->  [p, t, d]
    xg3 = xg[:, :].rearrange("(t p) d -> p t d", p=128)
    xa3 = xa[:, :].rearrange("(t p) d -> p t d", p=128)
    ox3 = ox[:, :].rearrange("(t p) d -> p t d", p=128)

    n_echunks = ETC // TC

    with TileContext(nc) as tc:
        with (
            tc.tile_pool(name="const", bufs=1) as cpool,
            tc.tile_pool(name="edge", bufs=3) as epool,
            tc.tile_pool(name="misc", bufs=1) as mpool,
            tc.tile_pool(name="wbat", bufs=4) as wpool,
            tc.tile_pool(name="fuse", bufs=3) as fpool,
            tc.tile_pool(name="ph", bufs=1, space="PSUM") as ppool,
            tc.tile_pool(name="ps", bufs=3, space="PSUM") as pspool,
            tc.tile_pool(name="dram", bufs=1, space="DRAM") as dpool,
        ):
            # ---- constants ----
            iota64f = cpool.tile([128, 64], F32)
            nc.gpsimd.iota(iota64f[:], [[1, 64]], channel_multiplier=0,
                           allow_small_or_imprecise_dtypes=True)
            ident = cpool.tile([128, 128], F32)
            make_identity(nc, ident[:])
            bias_m7 = cpool.tile([128, 1], F32)
            nc.vector.memset(bias_m7[:], -7.0)
            bias_m3 = cpool.tile([128, 1], F32)
            nc.vector.memset(bias_m3[:], -3.0)
            bias_m4 = cpool.tile([128, 1], F32)
            nc.vector.memset(bias_m4[:], -4.0)
            bias_p1 = cpool.tile([128, 1], F32)
            nc.vector.memset(bias_p1[:], 1.0)
            abgt = mpool.tile([8, 3], F32)
            nc.gpsimd.dma_start(out=abgt[:], in_=abg[:, :])
            # DVE-local copy so scalar APs don't add cross-engine sync waits
            abgs = mpool.tile([8, 3], F32)
            nc.vector.tensor_copy(abgs[:], abgt[:])

            # packed histogram accumulator: edge hist at free 0:32, node 32:64
            ph = ppool.tile([128, 16], F32)

            # ---- edge histogram (keys arrive pre-split: hi3=key>>3, lo3=key&7) ----
            # One-hot rows built per j as tensor_scalar(is_equal, imm j) in a
            # j-major [128, 8, TC] layout: every operand keeps inner step 1 so
            # the DVE runs 2x; rows are spread across DVE / GPSIMD / ACT.
            for c in range(n_echunks):
                sl = slice(c * TC, (c + 1) * TC)
                ekpt = epool.tile([128, TC], I16, tag="ekpt")
                nc.sync.dma_start(out=ekpt[:], in_=ekp[:, sl])
                ekft = epool.tile([128, TC], I16, tag="ekft")
                nc.sync.dma_start(out=ekft[:], in_=ekf[:, sl])
                edt = epool.tile([128, TC], I16, tag="edt")
                nc.sync.dma_start(out=edt[:], in_=ed[:, sl])

                # edm = dst_graph - src_graph (0 iff intra-graph edge):
                # hi' = hi + 8*(edm != 0) pushes masked edges out of range
                mm8 = epool.tile([128, TC], I16, tag="mm8")
                nc.vector.tensor_scalar(mm8[:], edt[:], 0, 8, AOT.not_equal,
                                        AOT.mult)
                gpp = epool.tile([128, TC], I16, tag="gpp")
                nc.vector.tensor_tensor(gpp[:], ekpt[:], mm8[:], AOT.add)

                U3 = epool.tile([128, 8, TC], BF16, tag="U3")
                V3 = epool.tile([128, 8, TC], BF16, tag="V3")
                # one-hot rows: 10 on DVE (2x tensor_scalar), 3 on ACT
                # (relu(1-|x-j|)), 3 on GPSIMD
                for j in range(8):
                    if j < 7:
                        nc.vector.tensor_scalar(U3[:, j, :], gpp[:], j, None,
                                                AOT.is_equal)
                    else:
                        t_abs = epool.tile([128, TC], F32, tag="t_abs")
                        nc.scalar.activation(t_abs[:], gpp[:], ACTF.Abs,
                                             bias=bias_m7[:], scale=1.0)
                        nc.scalar.activation(U3[:, j, :], t_abs[:], ACTF.Relu,
                                             bias=bias_p1[:], scale=-1.0)
                    if j < 3:
                        nc.gpsimd.tensor_scalar(V3[:, j, :], ekft[:], j, None,
                                                AOT.is_equal)
                    elif j == 3:
                        t_abs2 = epool.tile([128, TC], F32, tag="t_abs2")
                        nc.scalar.activation(t_abs2[:], ekft[:], ACTF.Abs,
                                             bias=bias_m3[:], scale=1.0)
                        nc.scalar.activation(V3[:, j, :], t_abs2[:], ACTF.Relu,
                                             bias=bias_p1[:], scale=-1.0)
                    else:
                        nc.vector.tensor_scalar(V3[:, j, :], ekft[:], j, None,
                                                AOT.is_equal)
                _emit_hist_mms(nc, ph, U3, V3, TC, c * TC, ETC, 0)

            # ---- node histogram (same machinery; all nodes count) ----
            gpn = mpool.tile([128, NT], I16)
            nc.sync.dma_start(out=gpn[:], in_=nkp[:, :])
            gfn = mpool.tile([128, NT], I16)
            nc.sync.dma_start(out=gfn[:], in_=nkf8[:, :])
            nkfull = mpool.tile([128, NT], I16)
            nc.vector.scalar_tensor_tensor(nkfull[:], in0=gpn[:], scalar=8,
                                           in1=gfn[:], op0=AOT.mult, op1=AOT.add)
            nkf = mpool.tile([128, NT], F32)
            nc.vector.tensor_scalar(nkf[:], nkfull[:], 0.0, None, AOT.add)
            Un = mpool.tile([128, 8, NT], BF16)
            Vn = mpool.tile([128, 8, NT], BF16)
            for j in range(8):
                nc.vector.tensor_scalar(Un[:, j, :], gpn[:], j, None,
                                        AOT.is_equal)
                nc.gpsimd.tensor_scalar(Vn[:, j, :], gfn[:], j, None,
                                        AOT.is_equal)
            _emit_hist_mms(nc, ph, Un, Vn, NT, 0, NT, 8)

            # ---- combine histograms (PSUM reads must be 32-part aligned:
            #      copy whole accumulator to SBUF first) ----
            phs = mpool.tile([128, 16], F32)
            nc.vector.tensor_scalar(phs[:], ph[:], 0.0, None, AOT.add)
            he = _combine_hist(nc, mpool, phs, 0)   # n_edges  [8,8]
            hn = _combine_hist(nc, mpool, phs, 8)  # n_nodes  [8,8]

            # ---- per-graph gate math on [8,8] ----
            s1 = mpool.tile([8, 8], F32)
            nc.scalar.activation(s1[:], hn[:], ACTF.Ln, bias=1.0, scale=1.0)
            s2 = mpool.tile([8, 8], F32)
            nc.vector.tensor_scalar(s2[:], s1[:], float(INV_LOG_NORM), None,
                                    AOT.mult)
            den = mpool.tile([8, 8], F32)
            nc.vector.scalar_tensor_tensor(den[:], in0=hn[:], scalar=-1.0,
                                           in1=hn[:], op0=AOT.add, op1=AOT.mult)
            nc.vector.tensor_scalar(den[:], den[:], 1e-8, None, AOT.add)
            rec = mpool.tile([8, 8], F32)
            nc.vector.reciprocal(rec[:], den[:])
            dens = mpool.tile([8, 8], F32)
            nc.vector.tensor_tensor(dens[:], he[:], rec[:], AOT.mult)
            t3 = mpool.tile([8, 8], F32)
            nc.vector.tensor_scalar(t3[:], dens[:], abgs[:, 1:2], abgs[:, 2:3],
                                    AOT.mult, AOT.add)
            arg = mpool.tile([8, 8], F32)
            nc.vector.scalar_tensor_tensor(arg[:], in0=s2[:], scalar=abgs[:, 0:1],
                                           in1=t3[:], op0=AOT.mult, op1=AOT.add)
            wg2 = mpool.tile([8, 8], F32)
            nc.scalar.activation(wg2[:], arg[:], ACTF.Sigmoid)
            # stage through DVE so every downstream DMA waits on one engine sem
            wg2s = mpool.tile([8, 8], F32)
            nc.vector.tensor_scalar(wg2s[:], wg2[:], 0.0, None, AOT.add)
            wl2 = mpool.tile([8, 8], F32)
            nc.vector.tensor_scalar(wl2[:], wg2s[:], -1.0, 1.0, AOT.mult, AOT.add)
            nc.gpsimd.dma_start(out=owl[:, :], in_=wl2[:])
            nc.gpsimd.dma_start(out=owg[:, :], in_=wg2s[:])

            # ---- w vector on 128 partitions: W2col[g,0]=wl[g] (g<64),
            #      W2col[64+g,1]=wl[g]; bounce through DRAM to cross partitions
            w64d = dpool.tile([8, 8], F32)
            nc.gpsimd.dma_start(out=w64d[:], in_=wl2[:])
            w64flat = w64d[:, :].rearrange("a b -> (a b)").unsqueeze(1)
            Wv64 = mpool.tile([64, 1], F32)
            nc.gpsimd.dma_start(out=Wv64[:], in_=w64flat)
            # assemble block-column rhs entirely on DVE so downstream matmuls
            # wait on a single engine semaphore
            W2colS = mpool.tile([128, 2], F32)
            nc.vector.memset(W2colS[:], 0.0)
            nc.vector.tensor_scalar(W2colS[0:64, 0:1], Wv64[:], 0.0, None,
                                    AOT.add)
            nc.vector.tensor_scalar(W2colS[64:128, 1:2], Wv64[:], 0.0, None,
                                    AOT.add)

            # ---- per-node weights + fusion, interleaved so the fusion DMA
            #      stream starts as soon as the first weight columns exist ----
            Wl_cols = mpool.tile([128, NT], F32)
            Wg_cols = mpool.tile([128, NT], F32)

            def emit_wcol_batch(b):
                O2 = wpool.tile([128, 128], F32, tag="O2", name=f"O2_{b}")
                nc.vector.tensor_scalar(O2[:, 0:64], iota64f[:],
                                        nkf[:, 2 * b : 2 * b + 1], None,
                                        AOT.is_equal)
                nc.vector.tensor_scalar(O2[:, 64:128], iota64f[:],
                                        nkf[:, 2 * b + 1 : 2 * b + 2], None,
                                        AOT.is_equal)
                O2t = pspool.tile([128, 128], F32, tag="O2t", name=f"O2t_{b}")
                nc.tensor.transpose(O2t[:], O2[:], ident[:])
                O2tS = wpool.tile([128, 128], F32, tag="O2tS", name=f"O2tS_{b}")
                nc.scalar.copy(O2tS[:], O2t[:])
                wc2 = pspool.tile([128, 2], F32, tag="wc2", name=f"wc2_{b}")
                nc.tensor.matmul(wc2[:], O2tS[:], W2colS[:], start=True,
                                 stop=True)
                nc.vector.tensor_scalar(Wl_cols[:, 2 * b : 2 * b + 2], wc2[:],
                                        0.0, None, AOT.add)
                nc.vector.tensor_scalar(Wg_cols[:, 2 * b : 2 * b + 2], wc2[:],
                                        -1.0, 1.0, AOT.mult, AOT.add)

            assert NT % 4 == 0
            for s in range(NT // 4):
                emit_wcol_batch(2 * s)
                emit_wcol_batch(2 * s + 1)
                g4 = fpool.tile([128, 4, D], F32, tag="g4", bufs=10)
                nc.sync.dma_start(out=g4[:], in_=xg3[:, 4 * s : 4 * s + 4, :])
                a4 = fpool.tile([128, 4, D], F32, tag="a4", bufs=10)
                nc.sync.dma_start(out=a4[:], in_=xa3[:, 4 * s : 4 * s + 4, :])
                o4 = fpool.tile([128, 4, D], F32, tag="o4", bufs=6)
                for i in range(4):
                    t = 4 * s + i
                    t1 = fpool.tile([128, D], F32, tag="t1", bufs=6)
                    nc.scalar.mul(t1[:], g4[:, i, :], Wl_cols[:, t : t + 1])
                    t2 = fpool.tile([128, D], F32, tag="t2", bufs=6)
                    nc.vector.tensor_scalar(t2[:], a4[:, i, :],
                                            Wg_cols[:, t : t + 1], None,
                                            AOT.mult)
                    nc.vector.tensor_tensor(o4[:, i, :], t1[:], t2[:], AOT.add)
                nc.sync.dma_start(out=ox3[:, 4 * s : 4 * s + 4, :], in_=o4[:])

    nc.finalize()
    return nc


def _shard_inputs(x_ggnn, x_appnp, edge_index, batch, alpha, beta, gamma):
    """Host-side sharding: 64 graphs per core. Returns (in_maps, NT, ETC,
    per-core real node counts)."""
    batch = np.asarray(batch)
    x_ggnn = np.asarray(x_ggnn)
    x_appnp = np.asarray(x_appnp)
    edge_index = np.asarray(edge_index)

    b32 = batch.astype(np.int32, copy=False)
    # node boundaries per graph (batch is sorted)
    starts = np.searchsorted(b32, np.arange(BATCH + 1), side="left").astype(np.int64)
    core_node_lo = starts[np.arange(NCORES) * GPC]
    core_node_hi = starts[np.arange(NCORES) * GPC + GPC]
    n_per_core = (core_node_hi - core_node_lo).astype(np.int64)

    NT = int(math.ceil(n_per_core.max() / 512.0)) * 4  # node cols, %4==0
    NT = max(NT, 16)
    NPAD = NT * 128

    # edges: route to core owning src's graph
    src = edge_index[0].astype(np.int64, copy=False)
    dst = edge_index[1].astype(np.int64, copy=False)
    src_b = b32[src]
    dst_b = b32[dst]
    core_of_edge = (src_b >> 6).astype(np.int8)
    order = np.argsort(core_of_edge, kind="stable")
    src_b_s = src_b[order]
    dst_b_s = dst_b[order]
    e_counts = np.bincount(core_of_edge, minlength=NCORES).astype(np.int64)
    e_off = np.concatenate([[0], np.cumsum(e_counts)])

    ETC = int(math.ceil(e_counts.max() / (128.0 * TC))) * TC
    ETC = max(ETC, TC)
    EPAD = ETC * 128

    abg_arr = np.tile(
        np.array([[alpha, beta, gamma]], dtype=np.float32), (8, 1)
    ).astype(np.float32)

    in_maps = []
    for k in range(NCORES):
        lo, hi = int(core_node_lo[k]), int(core_node_hi[k])
        nreal = hi - lo

        xg_k = np.zeros((NPAD, D), dtype=np.float32)
        xg_k[:nreal] = x_ggnn[lo:hi]
        xa_k = np.zeros((NPAD, D), dtype=np.float32)
        xa_k[:nreal] = x_appnp[lo:hi]

        nk_flat = np.full(NPAD, PAD_KEY, dtype=np.int16)
        nk_flat[:nreal] = (b32[lo:hi] - GPC * k).astype(np.int16)
        # tile-major: nk2[p, t] = key(node 128*t + p); split into 3-bit digits
        nk2 = np.ascontiguousarray(nk_flat.reshape(NT, 128).T)
        nkp2 = (nk2 >> 3).astype(np.int16)
        nkf2 = (nk2 & 7).astype(np.int16)

        el, eh = int(e_off[k]), int(e_off[k + 1])
        ne = eh - el
        ek_flat = np.full(EPAD, PAD_KEY, dtype=np.int16)
        ek_flat[:ne] = (src_b_s[el:eh] - GPC * k).astype(np.int16)
        ed_flat = np.full(EPAD, 1, dtype=np.int16)  # nonzero => masked out
        ed_flat[:ne] = (dst_b_s[el:eh] - src_b_s[el:eh]).astype(np.int16)
        ek2 = ek_flat.reshape(128, ETC)  # partition-major; order irrelevant
        ed2 = ed_flat.reshape(128, ETC)
        ekp2 = (ek2 >> 3).astype(np.int16)
        ekf2 = (ek2 & 7).astype(np.int16)

        in_maps.append({
            "xg": xg_k, "xa": xa_k,
            "ekp": ekp2, "ekf": ekf2, "ed": ed2,
            "nkp": nkp2, "nkf8": nkf2,
            "abg": abg_arr,
        })
    return in_maps, NT, ETC, n_per_core


_KERNEL_CACHE = {}


def kernel(x_ggnn, x_appnp, edge_index, batch, alpha, beta, gamma,
           trace=False, tmpdir=None):
    in_maps, NT, ETC, n_per_core = _shard_inputs(
        x_ggnn, x_appnp, edge_index, batch, alpha, beta, gamma)

    key = (NT, ETC)
    if key not in _KERNEL_CACHE:
        _KERNEL_CACHE[key] = build_kernel(NT, ETC)
    nc = _KERNEL_CACHE[key]

    res = run_bass_kernel_spmd(
        nc, in_maps, core_ids=list(range(NCORES)), trace=trace, tmpdir=tmpdir)

    x_fused = np.empty((N_NODES, D), dtype=np.float32)
    w_local = np.empty(BATCH, dtype=np.float32)
    w_global = np.empty(BATCH, dtype=np.float32)
    row = 0
    for k in range(NCORES):
        r = res.results[k]
        nreal = int(n_per_core[k])
        x_fused[row : row + nreal] = r["ox"][:nreal]
        row += nreal
        w_local[k * GPC : (k + 1) * GPC] = r["owl"].reshape(-1)
        w_global[k * GPC : (k + 1) * GPC] = r["owg"].reshape(-1)
    assert row == N_NODES
    kernel.last_results = res
    return (x_fused, w_local, w_global)


# revision 45
# speedup vs baseline: 1.0232x; 1.0232x over previous
"""Trainium2 Bass kernel for ExplicitComplexityFusion (segment_reduce).

Computes, for a batch of B=512 graphs packed into N=200000 nodes (batch ids
sorted) and E=6400000 random edges:
    n_nodes[b], n_edges[b] (intra-graph edges), per-graph sigmoid gate
    w_global/w_local, and x_fused = w_local[batch]*x_ggnn + w_global[batch]*x_appnp.

Sharding (per the data-parallel-over-graphs scheme): 64 consecutive graphs per
core; node rows and edges (keyed by src's graph) are routed to the owning core
on the host, so all segment reductions and the fusion are core-local.

Device algorithm per core:
  - histogram of 64 local graph keys via one-hot(hi3)⊗one-hot(lo3) TensorE
    matmuls accumulated in PSUM (edges masked by src_graph==dst_graph),
  - per-graph gate math (ln / reciprocal / sigmoid) on [8,8] tiles,
  - per-node weight = transpose(one-hot64) @ w  on TensorE,
  - fusion via ACT per-partition scale + DVE fused multiply-add.
"""

import math
import os
import sys

import numpy as np

for _p in ("/opt/trn_rl_repo", "/opt/trn_rl_repo/concourse"):
    if os.path.isdir(_p) and _p not in sys.path:
        sys.path.insert(0, _p)

import concourse.bass as bass
import concourse.mybir as mybir
from concourse import bacc
from concourse.masks import make_identity
from concourse.bass_utils import run_bass_kernel_spmd
from concourse.tile import TileContext

F32 = mybir.dt.float32
I16 = mybir.dt.int16
I32 = mybir.dt.int32
BF16 = mybir.dt.bfloat16
AOT = mybir.AluOpType
ACTF = mybir.ActivationFunctionType

N_NODES = 200000
D = 256
N_EDGES = 6400000
BATCH = 512
MAX_NODES = 500
NCORES = 8
GPC = BATCH // NCORES  # graphs per core = 64
TC = 512  # edge columns per chunk
INV_LOG_NORM = 1.0 / math.log(MAX_NODES + 1)

PAD_KEY = 64  # out-of-range key: hi3 = 8 -> zero one-hot row -> no contribution


def _emit_hist_mms(nc, ph, U3, V3, ncols, col0, total_cols, free_off):
    """Accumulate sum_e onehot8(hi)⊗onehot8(lo) into PSUM `ph`.

    U3/V3: [128, ncols, 8] bf16 one-hot tiles (128 edges per column).
    Columns are spread over the 4 PE column-groups via tile_position so up
    to 4 matmuls run concurrently in disjoint 32-partition strips.
    """
    assert ncols % 4 == 0
    for t in range(ncols):
        col = col0 + t
        j = col % 4
        nc.tensor.matmul(
            ph[32 * j : 32 * j + 8, free_off : free_off + 8],
            U3[:, :, t],
            V3[:, :, t],
            start=(col < 4),
            stop=(col >= total_cols - 4),
            tile_position=(0, 32 * j),
        )


def _combine_hist(nc, pool, phs, free_off):
    """Sum the 4 column-group 8x8 blocks of the histogram (SBUF copy).

    Engine ops need equal base partitions for two SBUF inputs, so stage the
    four blocks at base partition 0 first.
    """
    stk = pool.tile([8, 4, 8], F32, tag=f"histstk{free_off}")
    for j in range(4):
        nc.vector.tensor_copy(stk[:, j, :],
                              phs[32 * j : 32 * j + 8, free_off : free_off + 8])
    acc = pool.tile([8, 8], F32, tag=f"histacc{free_off}")
    nc.vector.tensor_tensor(acc[:], stk[:, 0, :], stk[:, 1, :], AOT.add)
    nc.vector.tensor_tensor(acc[:], acc[:], stk[:, 2, :], AOT.add)
    nc.vector.tensor_tensor(acc[:], acc[:], stk[:, 3, :], AOT.add)
    return acc


def build_kernel(NT, ETC):
    """Build the SPMD Bass program. NT: node columns (nodes_pad=128*NT),
    ETC: edge columns (edges_pad=128*ETC). NT % 4 == 0, ETC % TC == 0."""
    # Bacc (not plain Bass): its finalize() legalizes sync waits (split into
    # EventSemaphore chains) -- walrus allows only one wait per instruction.
    nc = bacc.Bacc("TRN2", target_bir_lowering=False)

    xg = nc.dram_tensor("xg", [NT * 128, D], F32, kind="ExternalInput")
    xa = nc.dram_tensor("xa", [NT * 128, D], F32, kind="ExternalInput")
    ekp = nc.dram_tensor("ekp", [128, ETC], I16, kind="ExternalInput")
    ekf = nc.dram_tensor("ekf", [128, ETC], I16, kind="ExternalInput")
    nkp = nc.dram_tensor("nkp", [128, NT], I16, kind="ExternalInput")
    nkf8 = nc.dram_tensor("nkf8", [128, NT], I16, kind="ExternalInput")
    abg = nc.dram_tensor("abg", [8, 3], F32, kind="ExternalInput")
    ox = nc.dram_tensor("ox", [NT * 128, D], F32, kind="ExternalOutput")
    owl = nc.dram_tensor("owl", [8, 8], F32, kind="ExternalOutput")
    owg = nc.dram_tensor("owg", [8, 8], F32, kind="ExternalOutput")

    # [row, d] with row = 128*t + p  ->  [p, t, d]
    xg3 = xg[:, :].rearrange("(t p) d -> p t d", p=128)
    xa3 = xa[:, :].rearrange("(t p) d -> p t d", p=128)
    ox3 = ox[:, :].rearrange("(t p) d -> p t d", p=128)

    n_echunks = ETC // TC

    with TileContext(nc) as tc:
        with (
            tc.tile_pool(name="const", bufs=1) as cpool,
            tc.tile_pool(name="edge", bufs=3) as epool,
            tc.tile_pool(name="misc", bufs=1) as mpool,
            tc.tile_pool(name="wbat", bufs=4) as wpool,
            tc.tile_pool(name="fuse", bufs=3) as fpool,
            tc.tile_pool(name="ph", bufs=1, space="PSUM") as ppool,
            tc.tile_pool(name="ps", bufs=3, space="PSUM") as pspool,
            tc.tile_pool(name="dram", bufs=1, space="DRAM") as dpool,
        ):
            # ---- constants ----
            iota64f = cpool.tile([128, 64], F32)
            nc.gpsimd.iota(iota64f[:], [[1, 64]], channel_multiplier=0,
                           allow_small_or_imprecise_dtypes=True)
            ident = cpool.tile([128, 128], F32)
            make_identity(nc, ident[:])
            bias_m7 = cpool.tile([128, 1], F32)
            nc.vector.memset(bias_m7[:], -7.0)
            bias_m3 = cpool.tile([128, 1], F32)
            nc.vector.memset(bias_m3[:], -3.0)
            bias_m4 = cpool.tile([128, 1], F32)
            nc.vector.memset(bias_m4[:], -4.0)
            bias_p1 = cpool.tile([128, 1], F32)
            nc.vector.memset(bias_p1[:], 1.0)
            abgt = mpool.tile([8, 3], F32)
            nc.gpsimd.dma_start(out=abgt[:], in_=abg[:, :])
            # DVE-local copy so scalar APs don't add cross-engine sync waits
            abgs = mpool.tile([8, 3], F32)
            nc.vector.tensor_copy(abgs[:], abgt[:])

            # packed histogram accumulator: edge hist at free 0:32, node 32:64
            ph = ppool.tile([128, 16], F32)

            # ---- edge histogram (keys arrive pre-split: hi3=key>>3, lo3=key&7) ----
            # One-hot rows built per j as tensor_scalar(is_equal, imm j) in a
            # j-major [128, 8, TC] layout: every operand keeps inner step 1 so
            # the DVE runs 2x; rows are spread across DVE / GPSIMD / ACT.
            for c in range(n_echunks):
                sl = slice(c * TC, (c + 1) * TC)
                ekpt = epool.tile([128, TC], I16, tag="ekpt")
                nc.sync.dma_start(out=ekpt[:], in_=ekp[:, sl])
                ekft = epool.tile([128, TC], I16, tag="ekft")
                nc.sync.dma_start(out=ekft[:], in_=ekf[:, sl])
                # host pre-folds the intra-graph mask into the hi digit:
                # ekp = (key>>3) + 8*(src_graph != dst_graph), so cross-graph
                # and pad edges fall outside [0,8) and match no U row
                gpp = ekpt

                U3 = epool.tile([128, 8, TC], BF16, tag="U3")
                V3 = epool.tile([128, 8, TC], BF16, tag="V3")
                # one-hot rows: 10 on DVE (2x tensor_scalar), 3 on ACT
                # (relu(1-|x-j|)), 3 on GPSIMD
                for j in range(8):
                    if j < 7:
                        nc.vector.tensor_scalar(U3[:, j, :], gpp[:], j, None,
                                                AOT.is_equal)
                    else:
                        t_abs = epool.tile([128, TC], F32, tag="t_abs")
                        nc.scalar.activation(t_abs[:], gpp[:], ACTF.Abs,
                                             bias=bias_m7[:], scale=1.0)
                        nc.scalar.activation(U3[:, j, :], t_abs[:], ACTF.Relu,
                                             bias=bias_p1[:], scale=-1.0)
                    if j < 3:
                        nc.gpsimd.tensor_scalar(V3[:, j, :], ekft[:], j, None,
                                                AOT.is_equal)
                    elif j == 3:
                        t_abs2 = epool.tile([128, TC], F32, tag="t_abs2")
                        nc.scalar.activation(t_abs2[:], ekft[:], ACTF.Abs,
                                             bias=bias_m3[:], scale=1.0)
                        nc.scalar.activation(V3[:, j, :], t_abs2[:], ACTF.Relu,
                                             bias=bias_p1[:], scale=-1.0)
                    else:
                        nc.vector.tensor_scalar(V3[:, j, :], ekft[:], j, None,
                                                AOT.is_equal)
                _emit_hist_mms(nc, ph, U3, V3, TC, c * TC, ETC, 0)

            # ---- node histogram (same machinery; all nodes count) ----
            gpn = mpool.tile([128, NT], I16)
            nc.sync.dma_start(out=gpn[:], in_=nkp[:, :])
            gfn = mpool.tile([128, NT], I16)
            nc.sync.dma_start(out=gfn[:], in_=nkf8[:, :])
            nkfull = mpool.tile([128, NT], I16)
            nc.vector.scalar_tensor_tensor(nkfull[:], in0=gpn[:], scalar=8,
                                           in1=gfn[:], op0=AOT.mult, op1=AOT.add)
            nkf = mpool.tile([128, NT], F32)
            nc.vector.tensor_scalar(nkf[:], nkfull[:], 0.0, None, AOT.add)
            Un = mpool.tile([128, 8, NT], BF16)
            Vn = mpool.tile([128, 8, NT], BF16)
            for j in range(8):
                nc.vector.tensor_scalar(Un[:, j, :], gpn[:], j, None,
                                        AOT.is_equal)
                nc.gpsimd.tensor_scalar(Vn[:, j, :], gfn[:], j, None,
                                        AOT.is_equal)
            _emit_hist_mms(nc, ph, Un, Vn, NT, 0, NT, 8)

            # ---- combine histograms (PSUM reads must be 32-part aligned:
            #      copy whole accumulator to SBUF first) ----
            phs = mpool.tile([128, 16], F32)
            nc.vector.tensor_scalar(phs[:], ph[:], 0.0, None, AOT.add)
            he = _combine_hist(nc, mpool, phs, 0)   # n_edges  [8,8]
            hn = _combine_hist(nc, mpool, phs, 8)  # n_nodes  [8,8]

            # ---- per-graph gate math on [8,8] ----
            s1 = mpool.tile([8, 8], F32)
            nc.scalar.activation(s1[:], hn[:], ACTF.Ln, bias=1.0, scale=1.0)
            s2 = mpool.tile([8, 8], F32)
            nc.vector.tensor_scalar(s2[:], s1[:], float(INV_LOG_NORM), None,
                                    AOT.mult)
            den = mpool.tile([8, 8], F32)
            nc.vector.scalar_tensor_tensor(den[:], in0=hn[:], scalar=-1.0,
                                           in1=hn[:], op0=AOT.add, op1=AOT.mult)
            nc.vector.tensor_scalar(den[:], den[:], 1e-8, None, AOT.add)
            rec = mpool.tile([8, 8], F32)
            nc.vector.reciprocal(rec[:], den[:])
            dens = mpool.tile([8, 8], F32)
            nc.vector.tensor_tensor(dens[:], he[:], rec[:], AOT.mult)
            t3 = mpool.tile([8, 8], F32)
            nc.vector.tensor_scalar(t3[:], dens[:], abgs[:, 1:2], abgs[:, 2:3],
                                    AOT.mult, AOT.add)
            arg = mpool.tile([8, 8], F32)
            nc.vector.scalar_tensor_tensor(arg[:], in0=s2[:], scalar=abgs[:, 0:1],
                                           in1=t3[:], op0=AOT.mult, op1=AOT.add)
            wg2 = mpool.tile([8, 8], F32)
            nc.scalar.activation(wg2[:], arg[:], ACTF.Sigmoid)
            # stage through DVE so every downstream DMA waits on one engine sem
            wg2s = mpool.tile([8, 8], F32)
            nc.vector.tensor_scalar(wg2s[:], wg2[:], 0.0, None, AOT.add)
            wl2 = mpool.tile([8, 8], F32)
            nc.vector.tensor_scalar(wl2[:], wg2s[:], -1.0, 1.0, AOT.mult, AOT.add)
            nc.gpsimd.dma_start(out=owl[:, :], in_=wl2[:])
            nc.gpsimd.dma_start(out=owg[:, :], in_=wg2s[:])

            # ---- w vector on 128 partitions: W2col[g,0]=wl[g] (g<64),
            #      W2col[64+g,1]=wl[g]; bounce through DRAM to cross partitions
            w64d = dpool.tile([8, 8], F32)
            nc.gpsimd.dma_start(out=w64d[:], in_=wl2[:])
            w64flat = w64d[:, :].rearrange("a b -> (a b)").unsqueeze(1)
            Wv64 = mpool.tile([64, 1], F32)
            nc.gpsimd.dma_start(out=Wv64[:], in_=w64flat)
            # assemble block-column rhs entirely on DVE so downstream matmuls
            # wait on a single engine semaphore
            W2colS = mpool.tile([128, 2], F32)
            nc.vector.memset(W2colS[:], 0.0)
            nc.vector.tensor_scalar(W2colS[0:64, 0:1], Wv64[:], 0.0, None,
                                    AOT.add)
            nc.vector.tensor_scalar(W2colS[64:128, 1:2], Wv64[:], 0.0, None,
                                    AOT.add)

            # ---- per-node weights + fusion, interleaved so the fusion DMA
            #      stream starts as soon as the first weight columns exist ----
            Wl_cols = mpool.tile([128, NT], F32)
            Wg_cols = mpool.tile([128, NT], F32)

            def emit_wcol_batch(b):
                O2 = wpool.tile([128, 128], F32, tag="O2", name=f"O2_{b}")
                nc.vector.tensor_scalar(O2[:, 0:64], iota64f[:],
                                        nkf[:, 2 * b : 2 * b + 1], None,
                                        AOT.is_equal)
                nc.vector.tensor_scalar(O2[:, 64:128], iota64f[:],
                                        nkf[:, 2 * b + 1 : 2 * b + 2], None,
                                        AOT.is_equal)
                O2t = pspool.tile([128, 128], F32, tag="O2t", name=f"O2t_{b}")
                nc.tensor.transpose(O2t[:], O2[:], ident[:])
                O2tS = wpool.tile([128, 128], F32, tag="O2tS", name=f"O2tS_{b}")
                nc.scalar.copy(O2tS[:], O2t[:])
                wc2 = pspool.tile([128, 2], F32, tag="wc2", name=f"wc2_{b}")
                nc.tensor.matmul(wc2[:], O2tS[:], W2colS[:], start=True,
                                 stop=True)
                nc.vector.tensor_scalar(Wl_cols[:, 2 * b : 2 * b + 2], wc2[:],
                                        0.0, None, AOT.add)
                nc.vector.tensor_scalar(Wg_cols[:, 2 * b : 2 * b + 2], wc2[:],
                                        -1.0, 1.0, AOT.mult, AOT.add)

            assert NT % 4 == 0
            for s in range(NT // 4):
                emit_wcol_batch(2 * s)
                emit_wcol_batch(2 * s + 1)
                g4 = fpool.tile([128, 4, D], F32, tag="g4", bufs=10)
                nc.sync.dma_start(out=g4[:], in_=xg3[:, 4 * s : 4 * s + 4, :])
                a4 = fpool.tile([128, 4, D], F32, tag="a4", bufs=10)
                nc.sync.dma_start(out=a4[:], in_=xa3[:, 4 * s : 4 * s + 4, :])
                o4 = fpool.tile([128, 4, D], F32, tag="o4", bufs=6)
                for i in range(4):
                    t = 4 * s + i
                    t1 = fpool.tile([128, D], F32, tag="t1", bufs=6)
                    nc.scalar.mul(t1[:], g4[:, i, :], Wl_cols[:, t : t + 1])
                    t2 = fpool.tile([128, D], F32, tag="t2", bufs=6)
                    nc.vector.tensor_scalar(t2[:], a4[:, i, :],
                                            Wg_cols[:, t : t + 1], None,
                                            AOT.mult)
                    nc.vector.tensor_tensor(o4[:, i, :], t1[:], t2[:], AOT.add)
                nc.sync.dma_start(out=ox3[:, 4 * s : 4 * s + 4, :], in_=o4[:])

    nc.finalize()
    return nc


def _shard_inputs(x_ggnn, x_appnp, edge_index, batch, alpha, beta, gamma):
    """Host-side sharding: 64 graphs per core. Returns (in_maps, NT, ETC,
    per-core real node counts)."""
    batch = np.asarray(batch)
    x_ggnn = np.asarray(x_ggnn)
    x_appnp = np.asarray(x_appnp)
    edge_index = np.asarray(edge_index)

    b32 = batch.astype(np.int32, copy=False)
    # node boundaries per graph (batch is sorted)
    starts = np.searchsorted(b32, np.arange(BATCH + 1), side="left").astype(np.int64)
    core_node_lo = starts[np.arange(NCORES) * GPC]
    core_node_hi = starts[np.arange(NCORES) * GPC + GPC]
    n_per_core = (core_node_hi - core_node_lo).astype(np.int64)

    NT = int(math.ceil(n_per_core.max() / 512.0)) * 4  # node cols, %4==0
    NT = max(NT, 16)
    NPAD = NT * 128

    # edges: route to core owning src's graph
    src = edge_index[0].astype(np.int64, copy=False)
    dst = edge_index[1].astype(np.int64, copy=False)
    src_b = b32[src]
    dst_b = b32[dst]
    core_of_edge = (src_b >> 6).astype(np.int8)
    order = np.argsort(core_of_edge, kind="stable")
    src_b_s = src_b[order]
    dst_b_s = dst_b[order]
    e_counts = np.bincount(core_of_edge, minlength=NCORES).astype(np.int64)
    e_off = np.concatenate([[0], np.cumsum(e_counts)])

    ETC = int(math.ceil(e_counts.max() / (128.0 * TC))) * TC
    ETC = max(ETC, TC)
    EPAD = ETC * 128

    abg_arr = np.tile(
        np.array([[alpha, beta, gamma]], dtype=np.float32), (8, 1)
    ).astype(np.float32)

    in_maps = []
    for k in range(NCORES):
        lo, hi = int(core_node_lo[k]), int(core_node_hi[k])
        nreal = hi - lo

        xg_k = np.zeros((NPAD, D), dtype=np.float32)
        xg_k[:nreal] = x_ggnn[lo:hi]
        xa_k = np.zeros((NPAD, D), dtype=np.float32)
        xa_k[:nreal] = x_appnp[lo:hi]

        nk_flat = np.full(NPAD, PAD_KEY, dtype=np.int16)
        nk_flat[:nreal] = (b32[lo:hi] - GPC * k).astype(np.int16)
        # tile-major: nk2[p, t] = key(node 128*t + p); split into 3-bit digits
        nk2 = np.ascontiguousarray(nk_flat.reshape(NT, 128).T)
        nkp2 = (nk2 >> 3).astype(np.int16)
        nkf2 = (nk2 & 7).astype(np.int16)

        el, eh = int(e_off[k]), int(e_off[k + 1])
        ne = eh - el
        ek_flat = np.full(EPAD, PAD_KEY, dtype=np.int16)
        ek_flat[:ne] = (src_b_s[el:eh] - GPC * k).astype(np.int16)
        cross = np.ones(EPAD, dtype=np.int16)  # pads count as cross-graph
        cross[:ne] = (src_b_s[el:eh] != dst_b_s[el:eh]).astype(np.int16)
        ek2 = ek_flat.reshape(128, ETC)  # partition-major; order irrelevant
        ekp2 = ((ek2 >> 3) + 8 * cross.reshape(128, ETC)).astype(np.int16)
        ekf2 = (ek2 & 7).astype(np.int16)

        in_maps.append({
            "xg": xg_k, "xa": xa_k,
            "ekp": ekp2, "ekf": ekf2,
            "nkp": nkp2, "nkf8": nkf2,
            "abg": abg_arr,
        })
    return in_maps, NT, ETC, n_per_core


_KERNEL_CACHE = {}


def kernel(x_ggnn, x_appnp, edge_index, batch, alpha, beta, gamma,
           trace=False, tmpdir=None):
    in_maps, NT, ETC, n_per_core = _shard_inputs(
        x_ggnn, x_appnp, edge_index, batch, alpha, beta, gamma)

    key = (NT, ETC)
    if key not in _KERNEL_CACHE:
        _KERNEL_CACHE[key] = build_kernel(NT, ETC)
    nc = _KERNEL_CACHE[key]

    res = run_bass_kernel_spmd(
        nc, in_maps, core_ids=list(range(NCORES)), trace=trace, tmpdir=tmpdir)

    x_fused = np.empty((N_NODES, D), dtype=np.float32)
    w_local = np.empty(BATCH, dtype=np.float32)
    w_global = np.empty(BATCH, dtype=np.float32)
    row = 0
    for k in range(NCORES):
        r = res.results[k]
        nreal = int(n_per_core[k])
        x_fused[row : row + nreal] = r["ox"][:nreal]
        row += nreal
        w_local[k * GPC : (k + 1) * GPC] = r["owl"].reshape(-1)
        w_global[k * GPC : (k + 1) * GPC] = r["owg"].reshape(-1)
    assert row == N_NODES
    kernel.last_results = res
    return (x_fused, w_local, w_global)


# revision 50
# speedup vs baseline: 1.0319x; 1.0085x over previous
"""Trainium2 Bass kernel for ExplicitComplexityFusion (segment_reduce).

Computes, for a batch of B=512 graphs packed into N=200000 nodes (batch ids
sorted) and E=6400000 random edges:
    n_nodes[b], n_edges[b] (intra-graph edges), per-graph sigmoid gate
    w_global/w_local, and x_fused = w_local[batch]*x_ggnn + w_global[batch]*x_appnp.

Sharding (per the data-parallel-over-graphs scheme): 64 consecutive graphs per
core; node rows and edges (keyed by src's graph) are routed to the owning core
on the host, so all segment reductions and the fusion are core-local.

Device algorithm per core:
  - histogram of 64 local graph keys via one-hot(hi3)⊗one-hot(lo3) TensorE
    matmuls accumulated in PSUM (edges masked by src_graph==dst_graph),
  - per-graph gate math (ln / reciprocal / sigmoid) on [8,8] tiles,
  - per-node weight = transpose(one-hot64) @ w  on TensorE,
  - fusion via ACT per-partition scale + DVE fused multiply-add.
"""

import math
import os
import sys

import numpy as np

for _p in ("/opt/trn_rl_repo", "/opt/trn_rl_repo/concourse"):
    if os.path.isdir(_p) and _p not in sys.path:
        sys.path.insert(0, _p)

import concourse.bass as bass
import concourse.mybir as mybir
from concourse import bacc
from concourse.masks import make_identity
from concourse.bass_utils import run_bass_kernel_spmd
from concourse.tile import TileContext

F32 = mybir.dt.float32
I16 = mybir.dt.int16
I32 = mybir.dt.int32
BF16 = mybir.dt.bfloat16
AOT = mybir.AluOpType
ACTF = mybir.ActivationFunctionType

N_NODES = 200000
D = 256
N_EDGES = 6400000
BATCH = 512
MAX_NODES = 500
NCORES = 8
GPC = BATCH // NCORES  # graphs per core = 64
TC = 512  # edge columns per chunk
INV_LOG_NORM = 1.0 / math.log(MAX_NODES + 1)

PAD_KEY = 64  # out-of-range key: hi3 = 8 -> zero one-hot row -> no contribution


def _emit_hist_mms(nc, ph, U3, V3, ncols, col0, total_cols, free_off):
    """Accumulate sum_e onehot8(hi)⊗onehot8(lo) into PSUM `ph`.

    U3/V3: [128, ncols, 8] bf16 one-hot tiles (128 edges per column).
    Columns are spread over the 4 PE column-groups via tile_position so up
    to 4 matmuls run concurrently in disjoint 32-partition strips.
    """
    assert ncols % 4 == 0
    for t in range(ncols):
        col = col0 + t
        j = col % 4
        nc.tensor.matmul(
            ph[32 * j : 32 * j + 8, free_off : free_off + 8],
            U3[:, :, t],
            V3[:, :, t],
            start=(col < 4),
            stop=(col >= total_cols - 4),
            tile_position=(0, 32 * j),
        )


def _combine_hist(nc, pool, phs, free_off):
    """Sum the 4 column-group 8x8 blocks of the histogram (SBUF copy).

    Engine ops need equal base partitions for two SBUF inputs, so stage the
    four blocks at base partition 0 first.
    """
    stk = pool.tile([8, 4, 8], F32, tag=f"histstk{free_off}")
    for j in range(4):
        nc.vector.tensor_copy(stk[:, j, :],
                              phs[32 * j : 32 * j + 8, free_off : free_off + 8])
    acc = pool.tile([8, 8], F32, tag=f"histacc{free_off}")
    nc.vector.tensor_tensor(acc[:], stk[:, 0, :], stk[:, 1, :], AOT.add)
    nc.vector.tensor_tensor(acc[:], acc[:], stk[:, 2, :], AOT.add)
    nc.vector.tensor_tensor(acc[:], acc[:], stk[:, 3, :], AOT.add)
    return acc


def build_kernel(NT, ETC):
    """Build the SPMD Bass program. NT: node columns (nodes_pad=128*NT),
    ETC: edge columns (edges_pad=128*ETC). NT % 4 == 0, ETC % TC == 0."""
    # Bacc (not plain Bass): its finalize() legalizes sync waits (split into
    # EventSemaphore chains) -- walrus allows only one wait per instruction.
    nc = bacc.Bacc("TRN2", target_bir_lowering=False)

    xg = nc.dram_tensor("xg", [NT * 128, D], F32, kind="ExternalInput")
    xa = nc.dram_tensor("xa", [NT * 128, D], F32, kind="ExternalInput")
    ekp = nc.dram_tensor("ekp", [128, ETC], I16, kind="ExternalInput")
    ekf = nc.dram_tensor("ekf", [128, ETC], I16, kind="ExternalInput")
    nkp = nc.dram_tensor("nkp", [128, NT], I16, kind="ExternalInput")
    nkf8 = nc.dram_tensor("nkf8", [128, NT], I16, kind="ExternalInput")
    abg = nc.dram_tensor("abg", [8, 3], F32, kind="ExternalInput")
    ox = nc.dram_tensor("ox", [NT * 128, D], F32, kind="ExternalOutput")
    owl = nc.dram_tensor("owl", [8, 8], F32, kind="ExternalOutput")
    owg = nc.dram_tensor("owg", [8, 8], F32, kind="ExternalOutput")

    # [row, d] with row = 128*t + p  ->  [p, t, d]
    xg3 = xg[:, :].rearrange("(t p) d -> p t d", p=128)
    xa3 = xa[:, :].rearrange("(t p) d -> p t d", p=128)
    ox3 = ox[:, :].rearrange("(t p) d -> p t d", p=128)

    n_echunks = ETC // TC

    with TileContext(nc) as tc:
        with (
            tc.tile_pool(name="const", bufs=1) as cpool,
            tc.tile_pool(name="edge", bufs=3) as epool,
            tc.tile_pool(name="misc", bufs=1) as mpool,
            tc.tile_pool(name="wbat", bufs=4) as wpool,
            tc.tile_pool(name="fuse", bufs=3) as fpool,
            tc.tile_pool(name="ph", bufs=1, space="PSUM") as ppool,
            tc.tile_pool(name="ps", bufs=3, space="PSUM") as pspool,
            tc.tile_pool(name="dram", bufs=1, space="DRAM") as dpool,
        ):
            # ---- constants ----
            iota64f = cpool.tile([128, 64], F32)
            nc.gpsimd.iota(iota64f[:], [[1, 64]], channel_multiplier=0,
                           allow_small_or_imprecise_dtypes=True)
            ident = cpool.tile([128, 128], F32)
            make_identity(nc, ident[:])
            bias_m7 = cpool.tile([128, 1], F32)
            nc.vector.memset(bias_m7[:], -7.0)
            bias_m3 = cpool.tile([128, 1], F32)
            nc.vector.memset(bias_m3[:], -3.0)
            bias_m4 = cpool.tile([128, 1], F32)
            nc.vector.memset(bias_m4[:], -4.0)
            bias_p1 = cpool.tile([128, 1], F32)
            nc.vector.memset(bias_p1[:], 1.0)
            abgt = mpool.tile([8, 3], F32)
            nc.gpsimd.dma_start(out=abgt[:], in_=abg[:, :])
            # DVE-local copy so scalar APs don't add cross-engine sync waits
            abgs = mpool.tile([8, 3], F32)
            nc.vector.tensor_copy(abgs[:], abgt[:])

            # packed histogram accumulator: edge hist at free 0:32, node 32:64
            ph = ppool.tile([128, 16], F32)

            # ---- edge histogram (keys arrive pre-split: hi3=key>>3, lo3=key&7) ----
            # One-hot rows built per j as tensor_scalar(is_equal, imm j) in a
            # j-major [128, 8, TC] layout: every operand keeps inner step 1 so
            # the DVE runs 2x; rows are spread across DVE / GPSIMD / ACT.
            for c in range(n_echunks):
                sl = slice(c * TC, (c + 1) * TC)
                ekpt = epool.tile([128, TC], I16, tag="ekpt")
                nc.sync.dma_start(out=ekpt[:], in_=ekp[:, sl])
                ekft = epool.tile([128, TC], I16, tag="ekft")
                nc.sync.dma_start(out=ekft[:], in_=ekf[:, sl])
                # host pre-folds the intra-graph mask into the hi digit:
                # ekp = (key>>3) + 8*(src_graph != dst_graph), so cross-graph
                # and pad edges fall outside [0,8) and match no U row
                gpp = ekpt

                U3 = epool.tile([128, 8, TC], BF16, tag="U3")
                V3 = epool.tile([128, 8, TC], BF16, tag="V3")
                # one-hot rows: 10 on DVE (2x tensor_scalar), 3 on ACT
                # (relu(1-|x-j|)), 3 on GPSIMD
                for j in range(8):
                    if j < 7:
                        nc.vector.tensor_scalar(U3[:, j, :], gpp[:], j, None,
                                                AOT.is_equal)
                    else:
                        t_abs = epool.tile([128, TC], F32, tag="t_abs")
                        nc.scalar.activation(t_abs[:], gpp[:], ACTF.Abs,
                                             bias=bias_m7[:], scale=1.0)
                        nc.scalar.activation(U3[:, j, :], t_abs[:], ACTF.Relu,
                                             bias=bias_p1[:], scale=-1.0)
                    if j < 2:
                        nc.gpsimd.tensor_scalar(V3[:, j, :], ekft[:], j, None,
                                                AOT.is_equal)
                    elif j == 3:
                        t_abs2 = epool.tile([128, TC], F32, tag="t_abs2")
                        nc.scalar.activation(t_abs2[:], ekft[:], ACTF.Abs,
                                             bias=bias_m3[:], scale=1.0)
                        nc.scalar.activation(V3[:, j, :], t_abs2[:], ACTF.Relu,
                                             bias=bias_p1[:], scale=-1.0)
                    else:
                        nc.vector.tensor_scalar(V3[:, j, :], ekft[:], j, None,
                                                AOT.is_equal)
                _emit_hist_mms(nc, ph, U3, V3, TC, c * TC, ETC, 0)

            # ---- node histogram (same machinery; all nodes count) ----
            gpn = mpool.tile([128, NT], I16)
            nc.sync.dma_start(out=gpn[:], in_=nkp[:, :])
            gfn = mpool.tile([128, NT], I16)
            nc.sync.dma_start(out=gfn[:], in_=nkf8[:, :])
            nkfull = mpool.tile([128, NT], I16)
            nc.vector.scalar_tensor_tensor(nkfull[:], in0=gpn[:], scalar=8,
                                           in1=gfn[:], op0=AOT.mult, op1=AOT.add)
            nkf = mpool.tile([128, NT], F32)
            nc.vector.tensor_scalar(nkf[:], nkfull[:], 0.0, None, AOT.add)
            Un = mpool.tile([128, 8, NT], BF16)
            Vn = mpool.tile([128, 8, NT], BF16)
            for j in range(8):
                nc.vector.tensor_scalar(Un[:, j, :], gpn[:], j, None,
                                        AOT.is_equal)
                nc.gpsimd.tensor_scalar(Vn[:, j, :], gfn[:], j, None,
                                        AOT.is_equal)
            _emit_hist_mms(nc, ph, Un, Vn, NT, 0, NT, 8)

            # ---- combine histograms (PSUM reads must be 32-part aligned:
            #      copy whole accumulator to SBUF first) ----
            phs = mpool.tile([128, 16], F32)
            nc.vector.tensor_scalar(phs[:], ph[:], 0.0, None, AOT.add)
            he = _combine_hist(nc, mpool, phs, 0)   # n_edges  [8,8]
            hn = _combine_hist(nc, mpool, phs, 8)  # n_nodes  [8,8]

            # ---- per-graph gate math on [8,8] ----
            s1 = mpool.tile([8, 8], F32)
            nc.scalar.activation(s1[:], hn[:], ACTF.Ln, bias=1.0, scale=1.0)
            s2 = mpool.tile([8, 8], F32)
            nc.vector.tensor_scalar(s2[:], s1[:], float(INV_LOG_NORM), None,
                                    AOT.mult)
            den = mpool.tile([8, 8], F32)
            nc.vector.scalar_tensor_tensor(den[:], in0=hn[:], scalar=-1.0,
                                           in1=hn[:], op0=AOT.add, op1=AOT.mult)
            nc.vector.tensor_scalar(den[:], den[:], 1e-8, None, AOT.add)
            rec = mpool.tile([8, 8], F32)
            nc.vector.reciprocal(rec[:], den[:])
            dens = mpool.tile([8, 8], F32)
            nc.vector.tensor_tensor(dens[:], he[:], rec[:], AOT.mult)
            t3 = mpool.tile([8, 8], F32)
            nc.vector.tensor_scalar(t3[:], dens[:], abgs[:, 1:2], abgs[:, 2:3],
                                    AOT.mult, AOT.add)
            arg = mpool.tile([8, 8], F32)
            nc.vector.scalar_tensor_tensor(arg[:], in0=s2[:], scalar=abgs[:, 0:1],
                                           in1=t3[:], op0=AOT.mult, op1=AOT.add)
            wg2 = mpool.tile([8, 8], F32)
            nc.scalar.activation(wg2[:], arg[:], ACTF.Sigmoid)
            # stage through DVE so every downstream DMA waits on one engine sem
            wg2s = mpool.tile([8, 8], F32)
            nc.vector.tensor_scalar(wg2s[:], wg2[:], 0.0, None, AOT.add)
            wl2 = mpool.tile([8, 8], F32)
            nc.vector.tensor_scalar(wl2[:], wg2s[:], -1.0, 1.0, AOT.mult, AOT.add)
            nc.gpsimd.dma_start(out=owl[:, :], in_=wl2[:])
            nc.gpsimd.dma_start(out=owg[:, :], in_=wg2s[:])

            # ---- w vector on 128 partitions: W2col[g,0]=wl[g] (g<64),
            #      W2col[64+g,1]=wl[g]; bounce through DRAM to cross partitions
            w64d = dpool.tile([8, 8], F32)
            nc.gpsimd.dma_start(out=w64d[:], in_=wl2[:])
            w64flat = w64d[:, :].rearrange("a b -> (a b)").unsqueeze(1)
            Wv64 = mpool.tile([64, 1], F32)
            nc.gpsimd.dma_start(out=Wv64[:], in_=w64flat)
            # assemble block-column rhs entirely on DVE so downstream matmuls
            # wait on a single engine semaphore
            W2colS = mpool.tile([128, 2], F32)
            nc.vector.memset(W2colS[:], 0.0)
            nc.vector.tensor_scalar(W2colS[0:64, 0:1], Wv64[:], 0.0, None,
                                    AOT.add)
            nc.vector.tensor_scalar(W2colS[64:128, 1:2], Wv64[:], 0.0, None,
                                    AOT.add)

            # ---- per-node weights + fusion, interleaved so the fusion DMA
            #      stream starts as soon as the first weight columns exist ----
            Wl_cols = mpool.tile([128, NT], F32)
            Wg_cols = mpool.tile([128, NT], F32)

            def emit_wcol_batch(b):
                O2 = wpool.tile([128, 128], F32, tag="O2", name=f"O2_{b}")
                nc.vector.tensor_scalar(O2[:, 0:64], iota64f[:],
                                        nkf[:, 2 * b : 2 * b + 1], None,
                                        AOT.is_equal)
                nc.vector.tensor_scalar(O2[:, 64:128], iota64f[:],
                                        nkf[:, 2 * b + 1 : 2 * b + 2], None,
                                        AOT.is_equal)
                O2t = pspool.tile([128, 128], F32, tag="O2t", name=f"O2t_{b}")
                nc.tensor.transpose(O2t[:], O2[:], ident[:])
                O2tS = wpool.tile([128, 128], F32, tag="O2tS", name=f"O2tS_{b}")
                nc.scalar.copy(O2tS[:], O2t[:])
                wc2 = pspool.tile([128, 2], F32, tag="wc2", name=f"wc2_{b}")
                nc.tensor.matmul(wc2[:], O2tS[:], W2colS[:], start=True,
                                 stop=True)
                nc.vector.tensor_scalar(Wl_cols[:, 2 * b : 2 * b + 2], wc2[:],
                                        0.0, None, AOT.add)
                nc.vector.tensor_scalar(Wg_cols[:, 2 * b : 2 * b + 2], wc2[:],
                                        -1.0, 1.0, AOT.mult, AOT.add)

            assert NT % 4 == 0
            for s in range(NT // 4):
                emit_wcol_batch(2 * s)
                emit_wcol_batch(2 * s + 1)
                g4 = fpool.tile([128, 4, D], F32, tag="g4", bufs=10)
                nc.sync.dma_start(out=g4[:], in_=xg3[:, 4 * s : 4 * s + 4, :])
                a4 = fpool.tile([128, 4, D], F32, tag="a4", bufs=10)
                nc.sync.dma_start(out=a4[:], in_=xa3[:, 4 * s : 4 * s + 4, :])
                o4 = fpool.tile([128, 4, D], F32, tag="o4", bufs=6)
                for i in range(4):
                    t = 4 * s + i
                    t1 = fpool.tile([128, D], F32, tag="t1", bufs=6)
                    nc.scalar.mul(t1[:], g4[:, i, :], Wl_cols[:, t : t + 1])
                    t2 = fpool.tile([128, D], F32, tag="t2", bufs=6)
                    nc.vector.tensor_scalar(t2[:], a4[:, i, :],
                                            Wg_cols[:, t : t + 1], None,
                                            AOT.mult)
                    nc.vector.tensor_tensor(o4[:, i, :], t1[:], t2[:], AOT.add)
                nc.sync.dma_start(out=ox3[:, 4 * s : 4 * s + 4, :], in_=o4[:])

    nc.finalize()
    return nc


def _shard_inputs(x_ggnn, x_appnp, edge_index, batch, alpha, beta, gamma):
    """Host-side sharding: 64 graphs per core. Returns (in_maps, NT, ETC,
    per-core real node counts)."""
    batch = np.asarray(batch)
    x_ggnn = np.asarray(x_ggnn)
    x_appnp = np.asarray(x_appnp)
    edge_index = np.asarray(edge_index)

    b32 = batch.astype(np.int32, copy=False)
    # node boundaries per graph (batch is sorted)
    starts = np.searchsorted(b32, np.arange(BATCH + 1), side="left").astype(np.int64)
    core_node_lo = starts[np.arange(NCORES) * GPC]
    core_node_hi = starts[np.arange(NCORES) * GPC + GPC]
    n_per_core = (core_node_hi - core_node_lo).astype(np.int64)

    NT = int(math.ceil(n_per_core.max() / 512.0)) * 4  # node cols, %4==0
    NT = max(NT, 16)
    NPAD = NT * 128

    # edges: route to core owning src's graph
    src = edge_index[0].astype(np.int64, copy=False)
    dst = edge_index[1].astype(np.int64, copy=False)
    src_b = b32[src]
    dst_b = b32[dst]
    core_of_edge = (src_b >> 6).astype(np.int8)
    order = np.argsort(core_of_edge, kind="stable")
    src_b_s = src_b[order]
    dst_b_s = dst_b[order]
    e_counts = np.bincount(core_of_edge, minlength=NCORES).astype(np.int64)
    e_off = np.concatenate([[0], np.cumsum(e_counts)])

    ETC = int(math.ceil(e_counts.max() / (128.0 * TC))) * TC
    ETC = max(ETC, TC)
    EPAD = ETC * 128

    abg_arr = np.tile(
        np.array([[alpha, beta, gamma]], dtype=np.float32), (8, 1)
    ).astype(np.float32)

    in_maps = []
    for k in range(NCORES):
        lo, hi = int(core_node_lo[k]), int(core_node_hi[k])
        nreal = hi - lo

        xg_k = np.zeros((NPAD, D), dtype=np.float32)
        xg_k[:nreal] = x_ggnn[lo:hi]
        xa_k = np.zeros((NPAD, D), dtype=np.float32)
        xa_k[:nreal] = x_appnp[lo:hi]

        nk_flat = np.full(NPAD, PAD_KEY, dtype=np.int16)
        nk_flat[:nreal] = (b32[lo:hi] - GPC * k).astype(np.int16)
        # tile-major: nk2[p, t] = key(node 128*t + p); split into 3-bit digits
        nk2 = np.ascontiguousarray(nk_flat.reshape(NT, 128).T)
        nkp2 = (nk2 >> 3).astype(np.int16)
        nkf2 = (nk2 & 7).astype(np.int16)

        el, eh = int(e_off[k]), int(e_off[k + 1])
        ne = eh - el
        ek_flat = np.full(EPAD, PAD_KEY, dtype=np.int16)
        ek_flat[:ne] = (src_b_s[el:eh] - GPC * k).astype(np.int16)
        cross = np.ones(EPAD, dtype=np.int16)  # pads count as cross-graph
        cross[:ne] = (src_b_s[el:eh] != dst_b_s[el:eh]).astype(np.int16)
        ek2 = ek_flat.reshape(128, ETC)  # partition-major; order irrelevant
        ekp2 = ((ek2 >> 3) + 8 * cross.reshape(128, ETC)).astype(np.int16)
        ekf2 = (ek2 & 7).astype(np.int16)

        in_maps.append({
            "xg": xg_k, "xa": xa_k,
            "ekp": ekp2, "ekf": ekf2,
            "nkp": nkp2, "nkf8": nkf2,
            "abg": abg_arr,
        })
    return in_maps, NT, ETC, n_per_core


_KERNEL_CACHE = {}


def kernel(x_ggnn, x_appnp, edge_index, batch, alpha, beta, gamma,
           trace=False, tmpdir=None):
    in_maps, NT, ETC, n_per_core = _shard_inputs(
        x_ggnn, x_appnp, edge_index, batch, alpha, beta, gamma)

    key = (NT, ETC)
    if key not in _KERNEL_CACHE:
        _KERNEL_CACHE[key] = build_kernel(NT, ETC)
    nc = _KERNEL_CACHE[key]

    res = run_bass_kernel_spmd(
        nc, in_maps, core_ids=list(range(NCORES)), trace=trace, tmpdir=tmpdir)

    x_fused = np.empty((N_NODES, D), dtype=np.float32)
    w_local = np.empty(BATCH, dtype=np.float32)
    w_global = np.empty(BATCH, dtype=np.float32)
    row = 0
    for k in range(NCORES):
        r = res.results[k]
        nreal = int(n_per_core[k])
        x_fused[row : row + nreal] = r["ox"][:nreal]
        row += nreal
        w_local[k * GPC : (k + 1) * GPC] = r["owl"].reshape(-1)
        w_global[k * GPC : (k + 1) * GPC] = r["owg"].reshape(-1)
    assert row == N_NODES
    kernel.last_results = res
    return (x_fused, w_local, w_global)
